# revision 9
# baseline (speedup 1.0000x reference)
"""AutoCorrelationLayer kernel for 8 TRN2 NeuronCores.

Math (per reference): Q/K/V projections (D=2048, H=8 heads, DH=256),
circular cross-correlation along the head dim per (b,h,l) implemented as
DFT-matmuls (L==S==DH==256 so the FFT becomes dense 256x256 matmuls on the
TensorEngine), softmax over the correlation axis, time-delay aggregation
(per-(b,h) 256x256 matmul with V), output projection.

Distribution: pure data-parallel over batch (B=32 -> 4 batches/core, zero
collectives).  All compute in fp16 operands with fp32 PSUM accumulation
(validated ~2.8e-3 rel err vs fp32 reference).  Activations are staged
feature-major (contraction dim on partitions) via host-side transposes of the
input shards; weights are passed transposed for the same reason.
"""

import numpy as np

import concourse.bass as bass
import concourse.mybir as mybir
import concourse.tile as tile_mod
from concourse.tile import TileContext
from concourse.vector_clock import ScopedClock
from concourse.bass_utils import run_bass_kernel_spmd

F32 = mybir.dt.float32
F16 = mybir.dt.float16
AF = mybir.ActivationFunctionType
AX = mybir.AxisListType

B, L, D, H = 32, 256, 2048, 8
DH = D // H          # 256
NCORES = 8
BPC = B // NCORES    # 4 batches per core
T = BPC * L          # 1024 tokens per core
NHALF = 2
TH = T // NHALF      # 512 tokens per half
EC = D // 128        # 16 feature chunks
DC = D // 128        # 16 contraction chunks


def _patch_tile_drain():
    """This walrus build allows at most ONE semaphore wait per instruction;
    Tile's kernel-tail drain collects one wait per live semaphore on a single
    Drain.  Split the extras onto additional drain instructions."""
    if getattr(tile_mod.TileContext, "_drain_split_patched", False):
        return

    def _drain_and_barrier(self, tick_clock, wait_clock):
        nc = self.nc
        drain_inst = nc.sync.drain()
        wait_clock.add_sem_waits(
            drain_inst.ins, ScopedClock({None: tick_clock.global_clock})
        )
        si = drain_inst.ins.sync_info
        waits = list(si.on_wait) if si is not None and si.on_wait else []
        if len(waits) > 1:
            drain_inst.ins.sync_info = mybir.SyncInfo(
                on_wait=[waits[0]], on_update=list(si.on_update or [])
            )
            for w in waits[1:]:
                extra = nc.sync.drain()
                extra.ins.sync_info = mybir.SyncInfo(on_wait=[w], on_update=[])
        nc.all_engine_barrier()
        popped = nc._tile_sem_poison_stack.pop()
        assert popped is self._sem_poison
        nc.clear_and_free_semaphores(list(self.sems.allocated().values()))
        nc.all_engine_barrier()

    tile_mod.TileContext._drain_and_barrier = _drain_and_barrier
    tile_mod.TileContext._drain_split_patched = True


def _split_multiwaits(nc):
    """Walrus in this build rejects >1 semaphore wait per instruction.  Hoist
    extra waits onto standalone EventSemaphore NOPs inserted just before the
    offending instruction on the same engine (engines execute in order)."""
    uid = [0]
    for fn in nc.m.functions:
        for bb in fn.blocks:
            il = bb.instructions
            i = 0
            while i < len(il):
                inst = il[i]
                si = inst.sync_info
                waits = list(si.on_wait) if si is not None and si.on_wait else []
                if len(waits) > 1:
                    carriers = []
                    for w in waits[:-1]:
                        uid[0] += 1
                        es = mybir.InstEventSemaphore(
                            name=f"mwsplit_{uid[0]}",
                            engine=inst.engine,
                            ins=[], outs=[],
                            sync_info=mybir.SyncInfo(on_wait=[w], on_update=[]),
                        )
                        carriers.append(es)
                    inst.sync_info = mybir.SyncInfo(
                        on_wait=[waits[-1]], on_update=list(si.on_update or [])
                    )
                    il[i:i] = carriers
                    i += len(carriers)
                i += 1


def build_kernel():
    _patch_tile_drain()
    nc = bass.Bass()

    xq = nc.declare_dram_parameter("xq", [D, T], F32, isOutput=False)  # queries^T
    xk = nc.declare_dram_parameter("xk", [D, T], F32, isOutput=False)
    xv = nc.declare_dram_parameter("xv", [D, T], F32, isOutput=False)
    wq = nc.declare_dram_parameter("wq", [D, D], F32, isOutput=False)  # Wq^T [d,e]
    wk = nc.declare_dram_parameter("wk", [D, D], F32, isOutput=False)
    wv = nc.declare_dram_parameter("wv", [D, D], F32, isOutput=False)
    wo = nc.declare_dram_parameter("wo", [D, D], F32, isOutput=False)
    bq = nc.declare_dram_parameter("bq", [D], F32, isOutput=False)
    bk = nc.declare_dram_parameter("bk", [D], F32, isOutput=False)
    bv = nc.declare_dram_parameter("bv", [D], F32, isOutput=False)
    bo = nc.declare_dram_parameter("bo", [D], F32, isOutput=False)
    tmp = nc.declare_dram_parameter("temp", [H], F32, isOutput=False)
    dftc = nc.declare_dram_parameter("dftc", [4, DH, DH], F32, isOutput=False)
    idn = nc.declare_dram_parameter("idn", [128, 128], F32, isOutput=False)
    out = nc.declare_dram_parameter("out", [T, D], F32, isOutput=True)

    def bcast_ap(param, n):
        return bass.AP(tensor=param, offset=0, ap=[[0, 128], [1, n]])

    with TileContext(nc) as tc:
        import contextlib

        with contextlib.ExitStack() as ctx:
            consts = ctx.enter_context(tc.tile_pool(name="consts", bufs=1))
            stg = ctx.enter_context(tc.tile_pool(name="stg", bufs=3))
            wstg = ctx.enter_context(tc.tile_pool(name="wstg", bufs=4))
            persist = ctx.enter_context(tc.tile_pool(name="persist", bufs=1))
            small = ctx.enter_context(tc.tile_pool(name="small", bufs=8))

            # ---- constants ----
            ident16 = consts.tile([128, 128], F16)
            s = stg.tile([128, 128], F32, tag="stg_id")
            nc.sync.dma_start(out=s, in_=idn[:])
            nc.vector.tensor_copy(ident16, s)

            # DFT matrices as fp16: fwd C,S as [m%128, mc, f]; inv Ci,Si as [f%128, fc, n]
            dmats = []
            for i in range(4):
                t16 = consts.tile([128, 2, DH], F16, name=f"dmat{i}", tag=f"dmat{i}")
                for c in range(2):
                    sd = stg.tile([128, DH], F32, tag="stg_dft")
                    nc.sync.dma_start(out=sd, in_=dftc[i, c * 128:(c + 1) * 128, :])
                    nc.vector.tensor_copy(t16[:, c, :], sd)
                dmats.append(t16)
            C_sb, S_sb, Ci_sb, Si_sb = dmats

            # biases as per-partition columns [128, EC]
            bq_sb = consts.tile([128, EC], F32)
            bk_sb = consts.tile([128, EC], F32)
            bv_sb = consts.tile([128, EC], F32)
            for bsb, bpar in ((bq_sb, bq), (bk_sb, bk), (bv_sb, bv)):
                nc.sync.dma_start(out=bsb, in_=bpar[:].rearrange("(ec p) -> p ec", p=128))
            # bo broadcast across partitions [128, D] and 1/temp columns
            bo_bc = consts.tile([128, D], F32)
            nc.sync.dma_start(out=bo_bc, in_=bcast_ap(bo, D))
            temp_bc = consts.tile([128, H], F32)
            nc.sync.dma_start(out=temp_bc, in_=bcast_ap(tmp, H))
            tinv = consts.tile([128, H], F32)
            nc.vector.reciprocal(tinv, temp_bc)
            ntinv = consts.tile([128, H], F32)
            nc.vector.tensor_scalar_mul(ntinv, tinv, -1.0)

            # persistent across halves
            outf16 = persist.tile([128, EC, T], F16)      # Out_f^T [e, t]
            v16 = persist.tile([128, TH // 128, D], F16)  # V token-major, per half

            for half in range(NHALF):
                t0 = half * TH

                with tc.tile_pool(name="xpool", bufs=1) as xpool:
                    # ---------- V projection: V^T[e,t] = Wv^T.T @ Xv^T ----------
                    xv16 = xpool.tile([128, DC, TH], F16, tag="x16")
                    for dc in range(DC):
                        sx = stg.tile([128, TH], F32, tag="stg_x")
                        nc.sync.dma_start(out=sx, in_=xv[dc * 128:(dc + 1) * 128, t0:t0 + TH])
                        nc.vector.tensor_copy(xv16[:, dc, :], sx)
                    with tc.tile_pool(name="vtpool", bufs=1) as vtpool, \
                         tc.tile_pool(name="psV", bufs=3, space="PSUM") as psV, \
                         tc.tile_pool(name="psVT", bufs=2, space="PSUM") as psVT:
                        vt16 = vtpool.tile([128, EC, TH], F16)
                        for ec in range(EC):
                            ps = psV.tile([128, TH], F32, tag="ps_proj")
                            for dc in range(DC):
                                sw = wstg.tile([128, 128], F32, tag="stg_w")
                                nc.sync.dma_start(
                                    out=sw, in_=wv[dc * 128:(dc + 1) * 128, ec * 128:(ec + 1) * 128])
                                w16 = wstg.tile([128, 128], F16, tag="w16")
                                nc.vector.tensor_copy(w16, sw)
                                nc.tensor.matmul(ps, w16[:], xv16[:, dc, :],
                                                 start=(dc == 0), stop=(dc == DC - 1))
                            nc.vector.tensor_copy(vt16[:, ec, :], ps)
                        # transpose V^T -> token-major V16 (bias bv folded post-softmax)
                        for ec in range(EC):
                            for tck in range(TH // 128):
                                pst = psVT.tile([128, 128], F16, tag="ps_tr")
                                nc.tensor.transpose(pst, vt16[:, ec, tck * 128:(tck + 1) * 128], ident16[:])
                                nc.scalar.activation(v16[:, tck, ec * 128:(ec + 1) * 128], pst, AF.Identity)

                with tc.tile_pool(name="qkpool", bufs=1) as qkpool:
                    q16 = qkpool.tile([128, EC, TH], F16, tag="q16")
                    k16 = qkpool.tile([128, EC, TH], F16, tag="k16")
                    for (dst16, xpar, wpar, bsb) in ((q16, xq, wq, bq_sb), (k16, xk, wk, bk_sb)):
                        with tc.tile_pool(name="xqk", bufs=1) as xqk, \
                             tc.tile_pool(name="psP", bufs=3, space="PSUM") as psP:
                            x16 = xqk.tile([128, DC, TH], F16, tag="x16b")
                            for dc in range(DC):
                                sx = stg.tile([128, TH], F32, tag="stg_x")
                                nc.sync.dma_start(out=sx, in_=xpar[dc * 128:(dc + 1) * 128, t0:t0 + TH])
                                nc.vector.tensor_copy(x16[:, dc, :], sx)
                            for ec in range(EC):
                                ps = psP.tile([128, TH], F32, tag="ps_proj")
                                for dc in range(DC):
                                    sw = wstg.tile([128, 128], F32, tag="stg_w")
                                    nc.sync.dma_start(
                                        out=sw, in_=wpar[dc * 128:(dc + 1) * 128, ec * 128:(ec + 1) * 128])
                                    w16 = wstg.tile([128, 128], F16, tag="w16")
                                    nc.vector.tensor_copy(w16, sw)
                                    nc.tensor.matmul(ps, w16[:], x16[:, dc, :],
                                                     start=(dc == 0), stop=(dc == DC - 1))
                                nc.scalar.activation(dst16[:, ec, :], ps, AF.Identity,
                                                     bias=bsb[:, ec:ec + 1])

                    # ---------- per-head DFT correlation + softmax + TDA ----------
                    with tc.tile_pool(name="hpool", bufs=2) as hpool, \
                         tc.tile_pool(name="epool", bufs=4) as epool, \
                         tc.tile_pool(name="psD", bufs=2, space="PSUM") as psD, \
                         tc.tile_pool(name="psB", bufs=2, space="PSUM") as psB, \
                         tc.tile_pool(name="psT", bufs=1, space="PSUM") as psT, \
                         tc.tile_pool(name="psO", bufs=2, space="PSUM") as psO:
                        for h in range(H):
                            qr = hpool.tile([128, 2, TH], F16, tag="qr")
                            qi = hpool.tile([128, 2, TH], F16, tag="qi")
                            kr = hpool.tile([128, 2, TH], F16, tag="kr")
                            ki = hpool.tile([128, 2, TH], F16, tag="ki")
                            for dst, src16, mat in ((qr, q16, C_sb), (qi, q16, S_sb),
                                                    (kr, k16, C_sb), (ki, k16, S_sb)):
                                for fc in range(2):
                                    ps = psD.tile([128, TH], F32, tag="ps_dft")
                                    for mc in range(2):
                                        nc.tensor.matmul(
                                            ps, mat[:, mc, fc * 128:(fc + 1) * 128],
                                            src16[:, h * 2 + mc, :],
                                            start=(mc == 0), stop=(mc == 1))
                                    nc.vector.tensor_copy(dst[:, fc, :], ps)
                            pr = hpool.tile([128, 2, TH], F16, tag="pr")
                            pi = hpool.tile([128, 2, TH], F16, tag="pi")
                            tmp16 = hpool.tile([128, 2, TH], F16, tag="tmp16")
                            nc.vector.tensor_mul(pr, qr, kr)
                            nc.vector.tensor_mul(tmp16, qi, ki)
                            nc.vector.tensor_add(pr, pr, tmp16)
                            nc.vector.tensor_mul(pi, qi, kr)
                            nc.vector.tensor_mul(tmp16, qr, ki)
                            nc.vector.tensor_sub(pi, pi, tmp16)

                            et16 = hpool.tile([128, 2, TH], F16, tag="et16")
                            for tck in range(TH // 128):
                                psc = psB.tile([128, DH], F32, tag="ps_corr")
                                nc.tensor.matmul(psc, pr[:, 0, tck * 128:(tck + 1) * 128],
                                                 Ci_sb[:, 0, :], start=True, stop=False)
                                nc.tensor.matmul(psc, pr[:, 1, tck * 128:(tck + 1) * 128],
                                                 Ci_sb[:, 1, :], start=False, stop=False)
                                nc.tensor.matmul(psc, pi[:, 0, tck * 128:(tck + 1) * 128],
                                                 Si_sb[:, 0, :], start=False, stop=False)
                                nc.tensor.matmul(psc, pi[:, 1, tck * 128:(tck + 1) * 128],
                                                 Si_sb[:, 1, :], start=False, stop=True)
                                mx = small.tile([128, 1], F32, tag="mx")
                                nc.vector.reduce_max(mx, psc[:], axis=AX.X)
                                nbias = small.tile([128, 1], F32, tag="nbias")
                                nc.vector.tensor_scalar_mul(nbias, mx, ntinv[:, h:h + 1])
                                e16 = epool.tile([128, DH], F16, tag="e16")
                                ssum = small.tile([128, 1], F32, tag="ssum")
                                nc.scalar.activation(e16, psc[:], AF.Exp,
                                                     bias=nbias[:], scale=tinv[:, h:h + 1],
                                                     accum_out=ssum[:])
                                rinv = small.tile([128, 1], F32, tag="rinv")
                                nc.vector.reciprocal(rinv, ssum)
                                en16 = epool.tile([128, DH], F16, tag="en16")
                                nc.vector.tensor_scalar_mul(en16, e16, rinv)
                                for sc in range(2):
                                    pst = psT.tile([128, 128], F16, tag="ps_et")
                                    nc.tensor.transpose(pst, en16[:, sc * 128:(sc + 1) * 128], ident16[:])
                                    nc.vector.tensor_copy(et16[:, sc, tck * 128:(tck + 1) * 128], pst)
                            # TDA: Outf^T[i, t] += Vp[s,i].T @ E^T[s,t] per local batch
                            for b in range(TH // L):
                                for ic in range(2):
                                    pso = psO.tile([128, L], F32, tag="ps_tda")
                                    for sc in range(2):
                                        nc.tensor.matmul(
                                            pso,
                                            v16[:, b * 2 + sc, h * DH + ic * 128:h * DH + (ic + 1) * 128],
                                            et16[:, sc, b * L:(b + 1) * L],
                                            start=(sc == 0), stop=(sc == 1))
                                    nc.scalar.activation(
                                        outf16[:, h * 2 + ic, t0 + b * L:t0 + (b + 1) * L],
                                        pso, AF.Identity, bias=bv_sb[:, h * 2 + ic:h * 2 + ic + 1])

            # ---------- output projection: Y[t,o] = Outf^T.T @ Wo^T + bo ----------
            with tc.tile_pool(name="wopool", bufs=1) as wopool, \
                 tc.tile_pool(name="ypool", bufs=4) as ypool, \
                 tc.tile_pool(name="psY", bufs=8, space="PSUM") as psY:
                wo16 = wopool.tile([128, EC, D], F16)
                for ec in range(EC):
                    sw = wopool.tile([128, D], F32, tag="stg_wo", bufs=2, name=f"stg_wo_{ec}")
                    nc.sync.dma_start(out=sw, in_=wo[ec * 128:(ec + 1) * 128, :])
                    nc.vector.tensor_copy(wo16[:, ec, :], sw)
                for tck in range(T // 128):
                    pss = [psY.tile([128, 512], F32, tag="ps_y", name=f"ps_y_{tck}_{i}")
                           for i in range(4)]
                    for ec in range(EC):
                        for oc in range(4):
                            nc.tensor.matmul(pss[oc], outf16[:, ec, tck * 128:(tck + 1) * 128],
                                             wo16[:, ec, oc * 512:(oc + 1) * 512],
                                             start=(ec == 0), stop=(ec == EC - 1))
                    for oc in range(4):
                        yt = ypool.tile([128, 512], F32, tag="yt")
                        nc.vector.tensor_add(yt, pss[oc], bo_bc[:, oc * 512:(oc + 1) * 512])
                        nc.sync.dma_start(out=out[tck * 128:(tck + 1) * 128, oc * 512:(oc + 1) * 512],
                                          in_=yt)
    _split_multiwaits(nc)
    return nc


_NC_CACHE = None


def _get_nc():
    global _NC_CACHE
    if _NC_CACHE is None:
        _NC_CACHE = build_kernel()
    return _NC_CACHE


def _dft_consts():
    m = np.arange(DH, dtype=np.float64)
    ang = 2.0 * np.pi * np.outer(m, m) / DH
    C = np.cos(ang)
    S = -np.sin(ang)
    Ci = np.cos(ang) / DH
    Si = -np.sin(ang) / DH
    return np.stack([C, S, Ci, Si]).astype(np.float32)


def make_in_maps(inputs):
    dftc = _dft_consts()
    idn = np.eye(128, dtype=np.float32)
    shared = {
        "wq": np.ascontiguousarray(inputs["Wq"].T).astype(np.float32, copy=False),
        "wk": np.ascontiguousarray(inputs["Wk"].T).astype(np.float32, copy=False),
        "wv": np.ascontiguousarray(inputs["Wv"].T).astype(np.float32, copy=False),
        "wo": np.ascontiguousarray(inputs["Wo"].T).astype(np.float32, copy=False),
        "bq": np.asarray(inputs["bq"], np.float32),
        "bk": np.asarray(inputs["bk"], np.float32),
        "bv": np.asarray(inputs["bv"], np.float32),
        "bo": np.asarray(inputs["bo"], np.float32),
        "temp": np.ascontiguousarray(np.asarray(inputs["temperature"], np.float32).reshape(H)),
        "dftc": dftc,
        "idn": idn,
    }
    in_maps = []
    for c in range(NCORES):
        sl = slice(c * BPC, (c + 1) * BPC)
        m = dict(shared)
        for key, name in (("queries", "xq"), ("keys", "xk"), ("values", "xv")):
            x = np.asarray(inputs[key], np.float32)[sl].reshape(T, D)
            m[name] = np.ascontiguousarray(x.T)
        in_maps.append(m)
    return in_maps


def kernel(**inputs):
    nc = _get_nc()
    in_maps = make_in_maps(inputs)
    res = run_bass_kernel_spmd(nc, in_maps, list(range(NCORES)))
    outs = [res.results[i]["out"].reshape(BPC, L, D) for i in range(NCORES)]
    return np.concatenate(outs, axis=0).astype(np.float32, copy=False)


# revision 10
# speedup vs baseline: 1.8799x; 1.8799x over previous
"""AutoCorrelationLayer kernel for 8 TRN2 NeuronCores.

Math (per reference): Q/K/V projections (D=2048, H=8 heads, DH=256),
circular cross-correlation along the head dim per (b,h,l) implemented as
DFT-matmuls (L==S==DH==256 so the FFT becomes dense 256x256 matmuls on the
TensorEngine), softmax over the correlation axis, time-delay aggregation
(per-(b,h) 256x256 matmul with V), output projection.

Distribution: pure data-parallel over batch (B=32 -> 4 batches/core, zero
collectives).  All compute in fp16 operands with fp32 PSUM accumulation
(validated ~2.8e-3 rel err vs fp32 reference).  Activations are staged
feature-major (contraction dim on partitions) via host-side transposes of the
input shards; weights are passed transposed for the same reason.
"""

import numpy as np

import concourse.bass as bass
import concourse.mybir as mybir
import concourse.tile as tile_mod
from concourse.tile import TileContext
from concourse.vector_clock import ScopedClock
from concourse.bass_utils import run_bass_kernel_spmd

F32 = mybir.dt.float32
F16 = mybir.dt.float16
AF = mybir.ActivationFunctionType
AX = mybir.AxisListType

B, L, D, H = 32, 256, 2048, 8
DH = D // H          # 256
NCORES = 8
BPC = B // NCORES    # 4 batches per core
T = BPC * L          # 1024 tokens per core
NHALF = 2
TH = T // NHALF      # 512 tokens per half
EC = D // 128        # 16 feature chunks
DC = D // 128        # 16 contraction chunks


def _patch_tile_drain():
    """This walrus build allows at most ONE semaphore wait per instruction;
    Tile's kernel-tail drain collects one wait per live semaphore on a single
    Drain.  Split the extras onto additional drain instructions."""
    if getattr(tile_mod.TileContext, "_drain_split_patched", False):
        return

    def _drain_and_barrier(self, tick_clock, wait_clock):
        nc = self.nc
        drain_inst = nc.sync.drain()
        wait_clock.add_sem_waits(
            drain_inst.ins, ScopedClock({None: tick_clock.global_clock})
        )
        si = drain_inst.ins.sync_info
        waits = list(si.on_wait) if si is not None and si.on_wait else []
        if len(waits) > 1:
            drain_inst.ins.sync_info = mybir.SyncInfo(
                on_wait=[waits[0]], on_update=list(si.on_update or [])
            )
            for w in waits[1:]:
                extra = nc.sync.drain()
                extra.ins.sync_info = mybir.SyncInfo(on_wait=[w], on_update=[])
        nc.all_engine_barrier()
        popped = nc._tile_sem_poison_stack.pop()
        assert popped is self._sem_poison
        nc.clear_and_free_semaphores(list(self.sems.allocated().values()))
        nc.all_engine_barrier()

    tile_mod.TileContext._drain_and_barrier = _drain_and_barrier
    tile_mod.TileContext._drain_split_patched = True


def _split_multiwaits(nc):
    """Walrus in this build rejects >1 semaphore wait per instruction.  Hoist
    extra waits onto standalone EventSemaphore NOPs inserted just before the
    offending instruction on the same engine (engines execute in order)."""
    uid = [0]
    for fn in nc.m.functions:
        for bb in fn.blocks:
            il = bb.instructions
            i = 0
            while i < len(il):
                inst = il[i]
                si = inst.sync_info
                waits = list(si.on_wait) if si is not None and si.on_wait else []
                if len(waits) > 1:
                    carriers = []
                    for w in waits[:-1]:
                        uid[0] += 1
                        es = mybir.InstEventSemaphore(
                            name=f"mwsplit_{uid[0]}",
                            engine=inst.engine,
                            ins=[], outs=[],
                            sync_info=mybir.SyncInfo(on_wait=[w], on_update=[]),
                        )
                        carriers.append(es)
                    inst.sync_info = mybir.SyncInfo(
                        on_wait=[waits[-1]], on_update=list(si.on_update or [])
                    )
                    il[i:i] = carriers
                    i += len(carriers)
                i += 1


def build_kernel():
    _patch_tile_drain()
    nc = bass.Bass()

    xq = nc.declare_dram_parameter("xq", [D, T], F32, isOutput=False)  # queries^T
    xk = nc.declare_dram_parameter("xk", [D, T], F32, isOutput=False)
    xv = nc.declare_dram_parameter("xv", [D, T], F32, isOutput=False)
    wq = nc.declare_dram_parameter("wq", [D, D], F32, isOutput=False)  # Wq^T [d,e]
    wk = nc.declare_dram_parameter("wk", [D, D], F32, isOutput=False)
    wv = nc.declare_dram_parameter("wv", [D, D], F32, isOutput=False)
    wo = nc.declare_dram_parameter("wo", [D, D], F32, isOutput=False)
    bq = nc.declare_dram_parameter("bq", [D], F32, isOutput=False)
    bk = nc.declare_dram_parameter("bk", [D], F32, isOutput=False)
    bv = nc.declare_dram_parameter("bv", [D], F32, isOutput=False)
    bo = nc.declare_dram_parameter("bo", [D], F32, isOutput=False)
    tmp = nc.declare_dram_parameter("temp", [H], F32, isOutput=False)
    dftc = nc.declare_dram_parameter("dftc", [4, DH, DH], F32, isOutput=False)
    idn = nc.declare_dram_parameter("idn", [128, 128], F32, isOutput=False)
    out = nc.declare_dram_parameter("out", [T, D], F32, isOutput=True)

    def bcast_ap(param, n):
        return bass.AP(tensor=param, offset=0, ap=[[0, 128], [1, n]])

    with TileContext(nc) as tc:
        import contextlib

        with contextlib.ExitStack() as ctx:
            consts = ctx.enter_context(tc.tile_pool(name="consts", bufs=1))
            stg = ctx.enter_context(tc.tile_pool(name="stg", bufs=4))
            wstg = ctx.enter_context(tc.tile_pool(name="wstg", bufs=6))
            persist = ctx.enter_context(tc.tile_pool(name="persist", bufs=1))
            small = ctx.enter_context(tc.tile_pool(name="small", bufs=8))

            # ---- constants ----
            ident16 = consts.tile([128, 128], F16)
            s = stg.tile([128, 128], F32, tag="stg_id")
            nc.sync.dma_start(out=s, in_=idn[:])
            nc.vector.tensor_copy(ident16, s)

            # DFT matrices as fp16: fwd C,S as [m%128, mc, f]; inv Ci,Si as [f%128, fc, n]
            dmats = []
            for i in range(4):
                t16 = consts.tile([128, 2, DH], F16, name=f"dmat{i}", tag=f"dmat{i}")
                for c in range(2):
                    sd = stg.tile([128, DH], F32, tag="stg_dft")
                    nc.sync.dma_start(out=sd, in_=dftc[i, c * 128:(c + 1) * 128, :])
                    nc.vector.tensor_copy(t16[:, c, :], sd)
                dmats.append(t16)
            C_sb, S_sb, Ci_sb, Si_sb = dmats

            # biases as per-partition columns [128, EC]
            bq_sb = consts.tile([128, EC], F32)
            bk_sb = consts.tile([128, EC], F32)
            bv_sb = consts.tile([128, EC], F32)
            for bsb, bpar in ((bq_sb, bq), (bk_sb, bk), (bv_sb, bv)):
                nc.sync.dma_start(out=bsb, in_=bpar[:].rearrange("(ec p) -> p ec", p=128))
            # bo broadcast across partitions [128, D] and 1/temp columns
            bo_bc = consts.tile([128, D], F32)
            nc.sync.dma_start(out=bo_bc, in_=bcast_ap(bo, D))
            temp_bc = consts.tile([128, H], F32)
            nc.sync.dma_start(out=temp_bc, in_=bcast_ap(tmp, H))
            tinv = consts.tile([128, H], F32)
            nc.vector.reciprocal(tinv, temp_bc)
            ntinv = consts.tile([128, H], F32)
            nc.vector.tensor_scalar_mul(ntinv, tinv, -1.0)

            # persistent across halves
            outf16 = persist.tile([128, EC, T], F16)      # Out_f^T [e, t]
            v16 = persist.tile([128, TH // 128, D], F16)  # V token-major, per half

            for half in range(NHALF):
                t0 = half * TH

                with tc.tile_pool(name="xpool", bufs=1) as xpool:
                    # ---------- V projection: V^T[e,t] = Wv^T.T @ Xv^T ----------
                    xv16 = xpool.tile([128, DC, TH], F16, tag="x16")
                    for dc in range(DC):
                        sx = stg.tile([128, TH], F32, tag="stg_x")
                        nc.sync.dma_start(out=sx, in_=xv[dc * 128:(dc + 1) * 128, t0:t0 + TH])
                        nc.vector.tensor_copy(xv16[:, dc, :], sx)
                    with tc.tile_pool(name="vtpool", bufs=1) as vtpool, \
                         tc.tile_pool(name="psV", bufs=6, space="PSUM") as psV, \
                         tc.tile_pool(name="psVT", bufs=2, space="PSUM") as psVT:
                        vt16 = vtpool.tile([128, EC, TH], F16)
                        for ecg in range(EC // 4):
                            pss = [psV.tile([128, TH], F32, tag="ps_proj",
                                            name=f"psv_{half}_{ecg}_{j}") for j in range(4)]
                            for dc in range(DC):
                                sw = wstg.tile([128, 512], F32, tag="stg_w")
                                nc.sync.dma_start(
                                    out=sw, in_=wv[dc * 128:(dc + 1) * 128, ecg * 512:(ecg + 1) * 512])
                                w16 = wstg.tile([128, 512], F16, tag="w16")
                                nc.vector.tensor_copy(w16, sw)
                                for j in range(4):
                                    nc.tensor.matmul(pss[j], w16[:, j * 128:(j + 1) * 128],
                                                     xv16[:, dc, :],
                                                     start=(dc == 0), stop=(dc == DC - 1))
                            for j in range(4):
                                nc.vector.tensor_copy(vt16[:, ecg * 4 + j, :], pss[j])
                        # transpose V^T -> token-major V16 (bias bv folded post-softmax)
                        for ec in range(EC):
                            for tck in range(TH // 128):
                                pst = psVT.tile([128, 128], F16, tag="ps_tr")
                                nc.tensor.transpose(pst, vt16[:, ec, tck * 128:(tck + 1) * 128], ident16[:])
                                nc.scalar.activation(v16[:, tck, ec * 128:(ec + 1) * 128], pst, AF.Identity)

                with tc.tile_pool(name="qkpool", bufs=1) as qkpool:
                    q16 = qkpool.tile([128, EC, TH], F16, tag="q16")
                    k16 = qkpool.tile([128, EC, TH], F16, tag="k16")
                    for (dst16, xpar, wpar, bsb) in ((q16, xq, wq, bq_sb), (k16, xk, wk, bk_sb)):
                        with tc.tile_pool(name="xqk", bufs=1) as xqk, \
                             tc.tile_pool(name="psP", bufs=8, space="PSUM") as psP:
                            x16 = xqk.tile([128, DC, TH], F16, tag="x16b")
                            for dc in range(DC):
                                sx = stg.tile([128, TH], F32, tag="stg_x")
                                nc.sync.dma_start(out=sx, in_=xpar[dc * 128:(dc + 1) * 128, t0:t0 + TH])
                                nc.vector.tensor_copy(x16[:, dc, :], sx)
                            for ecg in range(EC // 4):
                                pss = [psP.tile([128, TH], F32, tag="ps_proj",
                                                name=f"psp_{half}_{ecg}_{j}") for j in range(4)]
                                for dc in range(DC):
                                    sw = wstg.tile([128, 512], F32, tag="stg_w")
                                    nc.sync.dma_start(
                                        out=sw, in_=wpar[dc * 128:(dc + 1) * 128, ecg * 512:(ecg + 1) * 512])
                                    w16 = wstg.tile([128, 512], F16, tag="w16")
                                    nc.vector.tensor_copy(w16, sw)
                                    for j in range(4):
                                        nc.tensor.matmul(pss[j], w16[:, j * 128:(j + 1) * 128],
                                                         x16[:, dc, :],
                                                         start=(dc == 0), stop=(dc == DC - 1))
                                for j in range(4):
                                    ec = ecg * 4 + j
                                    nc.scalar.activation(dst16[:, ec, :], pss[j], AF.Identity,
                                                         bias=bsb[:, ec:ec + 1])

                    # ---------- per-head DFT correlation + softmax + TDA ----------
                    with tc.tile_pool(name="hpool", bufs=2) as hpool, \
                         tc.tile_pool(name="epool", bufs=4) as epool, \
                         tc.tile_pool(name="psD", bufs=2, space="PSUM") as psD, \
                         tc.tile_pool(name="psB", bufs=2, space="PSUM") as psB, \
                         tc.tile_pool(name="psT", bufs=1, space="PSUM") as psT, \
                         tc.tile_pool(name="psO", bufs=2, space="PSUM") as psO:
                        for h in range(H):
                            qr = hpool.tile([128, 2, TH], F16, tag="qr")
                            qi = hpool.tile([128, 2, TH], F16, tag="qi")
                            kr = hpool.tile([128, 2, TH], F16, tag="kr")
                            ki = hpool.tile([128, 2, TH], F16, tag="ki")
                            for dst, src16, mat in ((qr, q16, C_sb), (qi, q16, S_sb),
                                                    (kr, k16, C_sb), (ki, k16, S_sb)):
                                for fc in range(2):
                                    ps = psD.tile([128, TH], F32, tag="ps_dft")
                                    for mc in range(2):
                                        nc.tensor.matmul(
                                            ps, mat[:, mc, fc * 128:(fc + 1) * 128],
                                            src16[:, h * 2 + mc, :],
                                            start=(mc == 0), stop=(mc == 1))
                                    nc.vector.tensor_copy(dst[:, fc, :], ps)
                            pr = hpool.tile([128, 2, TH], F16, tag="pr")
                            pi = hpool.tile([128, 2, TH], F16, tag="pi")
                            tmp16 = hpool.tile([128, 2, TH], F16, tag="tmp16")
                            nc.vector.tensor_mul(pr, qr, kr)
                            nc.vector.tensor_mul(tmp16, qi, ki)
                            nc.vector.tensor_add(pr, pr, tmp16)
                            nc.vector.tensor_mul(pi, qi, kr)
                            nc.vector.tensor_mul(tmp16, qr, ki)
                            nc.vector.tensor_sub(pi, pi, tmp16)

                            et16 = hpool.tile([128, 2, TH], F16, tag="et16")
                            for tck in range(TH // 128):
                                psc = psB.tile([128, DH], F32, tag="ps_corr")
                                nc.tensor.matmul(psc, pr[:, 0, tck * 128:(tck + 1) * 128],
                                                 Ci_sb[:, 0, :], start=True, stop=False)
                                nc.tensor.matmul(psc, pr[:, 1, tck * 128:(tck + 1) * 128],
                                                 Ci_sb[:, 1, :], start=False, stop=False)
                                nc.tensor.matmul(psc, pi[:, 0, tck * 128:(tck + 1) * 128],
                                                 Si_sb[:, 0, :], start=False, stop=False)
                                nc.tensor.matmul(psc, pi[:, 1, tck * 128:(tck + 1) * 128],
                                                 Si_sb[:, 1, :], start=False, stop=True)
                                mx = small.tile([128, 1], F32, tag="mx")
                                nc.vector.reduce_max(mx, psc[:], axis=AX.X)
                                nbias = small.tile([128, 1], F32, tag="nbias")
                                nc.vector.tensor_scalar_mul(nbias, mx, ntinv[:, h:h + 1])
                                e16 = epool.tile([128, DH], F16, tag="e16")
                                ssum = small.tile([128, 1], F32, tag="ssum")
                                nc.scalar.activation(e16, psc[:], AF.Exp,
                                                     bias=nbias[:], scale=tinv[:, h:h + 1],
                                                     accum_out=ssum[:])
                                rinv = small.tile([128, 1], F32, tag="rinv")
                                nc.vector.reciprocal(rinv, ssum)
                                en16 = epool.tile([128, DH], F16, tag="en16")
                                nc.vector.tensor_scalar_mul(en16, e16, rinv)
                                for sc in range(2):
                                    pst = psT.tile([128, 128], F16, tag="ps_et")
                                    nc.tensor.transpose(pst, en16[:, sc * 128:(sc + 1) * 128], ident16[:])
                                    nc.vector.tensor_copy(et16[:, sc, tck * 128:(tck + 1) * 128], pst)
                            # TDA: Outf^T[i, t] += Vp[s,i].T @ E^T[s,t] per local batch
                            for b in range(TH // L):
                                for ic in range(2):
                                    pso = psO.tile([128, L], F32, tag="ps_tda")
                                    for sc in range(2):
                                        nc.tensor.matmul(
                                            pso,
                                            v16[:, b * 2 + sc, h * DH + ic * 128:h * DH + (ic + 1) * 128],
                                            et16[:, sc, b * L:(b + 1) * L],
                                            start=(sc == 0), stop=(sc == 1))
                                    nc.scalar.activation(
                                        outf16[:, h * 2 + ic, t0 + b * L:t0 + (b + 1) * L],
                                        pso, AF.Identity, bias=bv_sb[:, h * 2 + ic:h * 2 + ic + 1])

            # ---------- output projection: Y[t,o] = Outf^T.T @ Wo^T + bo ----------
            with tc.tile_pool(name="wopool", bufs=1) as wopool, \
                 tc.tile_pool(name="ypool", bufs=4) as ypool, \
                 tc.tile_pool(name="psY", bufs=8, space="PSUM") as psY:
                wo16 = wopool.tile([128, EC, D], F16)
                for ec in range(EC):
                    sw = wopool.tile([128, D], F32, tag="stg_wo", bufs=2, name=f"stg_wo_{ec}")
                    nc.sync.dma_start(out=sw, in_=wo[ec * 128:(ec + 1) * 128, :])
                    nc.vector.tensor_copy(wo16[:, ec, :], sw)
                for tck in range(T // 128):
                    pss = [psY.tile([128, 512], F32, tag="ps_y", name=f"ps_y_{tck}_{i}")
                           for i in range(4)]
                    for ec in range(EC):
                        for oc in range(4):
                            nc.tensor.matmul(pss[oc], outf16[:, ec, tck * 128:(tck + 1) * 128],
                                             wo16[:, ec, oc * 512:(oc + 1) * 512],
                                             start=(ec == 0), stop=(ec == EC - 1))
                    for oc in range(4):
                        yt = ypool.tile([128, 512], F32, tag="yt")
                        nc.vector.tensor_add(yt, pss[oc], bo_bc[:, oc * 512:(oc + 1) * 512])
                        nc.sync.dma_start(out=out[tck * 128:(tck + 1) * 128, oc * 512:(oc + 1) * 512],
                                          in_=yt)
    _split_multiwaits(nc)
    return nc


_NC_CACHE = None


def _get_nc():
    global _NC_CACHE
    if _NC_CACHE is None:
        _NC_CACHE = build_kernel()
    return _NC_CACHE


def _dft_consts():
    m = np.arange(DH, dtype=np.float64)
    ang = 2.0 * np.pi * np.outer(m, m) / DH
    C = np.cos(ang)
    S = -np.sin(ang)
    Ci = np.cos(ang) / DH
    Si = -np.sin(ang) / DH
    return np.stack([C, S, Ci, Si]).astype(np.float32)


def make_in_maps(inputs):
    dftc = _dft_consts()
    idn = np.eye(128, dtype=np.float32)
    shared = {
        "wq": np.ascontiguousarray(inputs["Wq"].T).astype(np.float32, copy=False),
        "wk": np.ascontiguousarray(inputs["Wk"].T).astype(np.float32, copy=False),
        "wv": np.ascontiguousarray(inputs["Wv"].T).astype(np.float32, copy=False),
        "wo": np.ascontiguousarray(inputs["Wo"].T).astype(np.float32, copy=False),
        "bq": np.asarray(inputs["bq"], np.float32),
        "bk": np.asarray(inputs["bk"], np.float32),
        "bv": np.asarray(inputs["bv"], np.float32),
        "bo": np.asarray(inputs["bo"], np.float32),
        "temp": np.ascontiguousarray(np.asarray(inputs["temperature"], np.float32).reshape(H)),
        "dftc": dftc,
        "idn": idn,
    }
    in_maps = []
    for c in range(NCORES):
        sl = slice(c * BPC, (c + 1) * BPC)
        m = dict(shared)
        for key, name in (("queries", "xq"), ("keys", "xk"), ("values", "xv")):
            x = np.asarray(inputs[key], np.float32)[sl].reshape(T, D)
            m[name] = np.ascontiguousarray(x.T)
        in_maps.append(m)
    return in_maps


def kernel(**inputs):
    nc = _get_nc()
    in_maps = make_in_maps(inputs)
    res = run_bass_kernel_spmd(nc, in_maps, list(range(NCORES)))
    outs = [res.results[i]["out"].reshape(BPC, L, D) for i in range(NCORES)]
    return np.concatenate(outs, axis=0).astype(np.float32, copy=False)
